# revision 10
# baseline (speedup 1.0000x reference)
"""Causal self-attention (B=4, S=2048, D=1024, single 1024-wide head) on 8 TRN2 cores.

Sharding: core c -> batch b=c//2, parity h=c%2. Queries: core h handles global
query blocks {h, h+2, ..., h+14} (128 rows each). Keys: core h PROJECTS K/V
only for its parity key tiles (half the work) and the pair exchanges halves
with chunked AllGather collectives (replica groups [[0,1],[2,3],[4,5],[6,7]]),
so the K^T/V projections are computed once per batch instead of twice.

Gathered layouts are rank-major == parity-major: KT_s cols [0:1024) hold
parity-0 key tiles 0..7, cols [1024:2048) parity-1 tiles; V_s slots 0-7 are
parity-0 tiles, 8-15 parity-1. Query block j of core h (global block g=2j+h)
attends parity-p tiles 0..j for both p — a uniform program on every core;
causality differences live in per-core additive-mask input data.

Own keys == own query rows for a core, so a single xoT input feeds both the
K/V projections (own keys) and the Q projection (own queries).

All matmuls run on the PE in bf16 with fp32 PSUM accumulation. Softmax skips
max-subtraction (scores are ~N(0,1); exp stays in fp32 range) so the
denominator comes free from the Exp activation's accumulate output. The P-tile
transposes needed for the AV matmul run on the DMA xbar (SBUF->SBUF), keeping
the PE free for matmuls and the DVE free for output normalization.
"""

import time

import numpy as np
import ml_dtypes

import concourse.bass as bass
import concourse.bacc as bacc
import concourse.tile as tile
from concourse import mybir
from concourse import bass_utils

BF16 = ml_dtypes.bfloat16
P = 128
B, S, D = 4, 2048, 1024
EC = D // P   # contraction chunks (8)
NQB = 8       # query blocks per core
NKB = S // P  # key blocks per batch (16)
NOT = NKB // 2  # own key tiles per core (8)
NCORES = 8
MASKV = -960.0  # additive pre-scale mask; -30 after the 1/sqrt(D) scale
CC_GROUPS = [[0, 1], [2, 3], [4, 5], [6, 7]]

_compiled_nc = None
_runner = None  # cached (sharded_jit, in_names, out_names, out_avals, n_params)
last_result = None  # kept for compatibility with older test harnesses


def _trace_kernel(tc, out, xoT, wqT, wkT, wvT, maskadd):
    nc = tc.nc
    f32 = mybir.dt.float32
    bf16 = mybir.dt.bfloat16
    ts = bass.ts

    with (
        tc.tile_pool(name="sb", bufs=1) as sb,
        tc.tile_pool(name="ps", bufs=2, space="PSUM") as ps,
        tc.tile_pool(name="dram", bufs=1, space="DRAM") as dram,
    ):
        # ---- persistent SBUF ----
        xoT_s = sb.tile([P, EC, D], bf16)   # own keys==queries columns of x^T
        KT_s = sb.tile([P, EC, S], bf16)    # gathered K^T, parity-major cols
        V_s = sb.tile([P, NKB, D], bf16)    # gathered V; slots 0-7 par0, 8-15 par1
        QT_s = sb.tile([P, EC, D], bf16)    # Q^T for own queries
        mask_s = sb.tile([P, NQB, 2 * P], f32)

        # ---- DRAM bounce buffers for the pair exchange ----
        kin = [dram.tile([P, EC, 512], bf16, name=f"kin{i}") for i in range(2)]
        kout = [dram.tile([2, P, EC, 512], bf16, name=f"kout{i}") for i in range(2)]
        vin = [dram.tile([P, 4, D], bf16, name=f"vin{i}") for i in range(2)]
        vout = [dram.tile([2, P, 4, D], bf16, name=f"vout{i}") for i in range(2)]

        def load_w(w_dram, nm):
            w_s = sb.tile([P, EC, D], bf16, tag="w", bufs=2, name=nm)
            # first 128-col slice lands first so the first dependent matmul
            # can issue before the bulk of the weight arrives
            nc.sync.dma_start(w_s[:, 0, :P], w_dram[:P, :P])
            nc.sync.dma_start(w_s[:, 0, P:], w_dram[:P, P:])
            for ec in range(1, EC):
                nc.sync.dma_start(w_s[:, ec], w_dram[ts(ec, P), :])
            return w_s

        # interleave the first projection's operands so PE starts ASAP
        wk_s = sb.tile([P, EC, D], bf16, tag="w", bufs=2, name="wk_s")
        nc.sync.dma_start(wk_s[:, 0, :P], wkT[:P, :P])
        nc.sync.dma_start(xoT_s[:, 0, :512], xoT[:P, :512])
        nc.sync.dma_start(wk_s[:, 0, P:], wkT[:P, P:])
        nc.sync.dma_start(xoT_s[:, 0, 512:], xoT[:P, 512:])
        for ec in range(1, EC):
            nc.sync.dma_start(wk_s[:, ec], wkT[ts(ec, P), :])
            nc.sync.dma_start(xoT_s[:, ec], xoT[ts(ec, P), :])
        for j in range(NQB):
            nc.sync.dma_start(mask_s[:, j], maskadd[j])

        # ---- interleaved projection phases + chunked pair exchange ----
        # CC stream order K1, V1, K2, V2 with projections K1p V1p Qa K2p V2p Qb
        # so every AllGather's input is staged just before the serial CC
        # stream frees up, and early attention blocks can start while the
        # tail chunks are still in flight.
        def k_proj_chunk(sc):
            # own keys [512sc, 512sc+512) = own tiles 4sc..4sc+3
            for dc in range(EC):
                acc = ps.tile([P, 512], f32, tag="s")
                for ec in range(EC):
                    nc.tensor.matmul(
                        acc, wk_s[:, ec, ts(dc, P)], xoT_s[:, ec, ts(sc, 512)],
                        start=(ec == 0), stop=(ec == EC - 1))
                stg = sb.tile([P, 512], bf16, tag="kstg", bufs=4)
                nc.scalar.copy(stg, acc)
                nc.sync.dma_start(kin[sc][:, dc, :], stg)
            nc.gpsimd.collective_compute(
                "AllGather", mybir.AluOpType.bypass,
                replica_groups=CC_GROUPS,
                ins=[kin[sc].opt()], outs=[kout[sc].opt()])
            for r in range(2):
                nc.sync.dma_start(
                    KT_s[:, :, r * 1024 + sc * 512: r * 1024 + sc * 512 + 512],
                    kout[sc][r])

        def v_proj_chunk(wv_s, ch):
            for ot in range(4 * ch, 4 * ch + 4):
                acc = ps.tile([P, D], f32, tag="big")
                for ec in range(EC):
                    lhsT = xoT_s[:, ec, ts(ot, P)]
                    for nh in range(2):
                        nc.tensor.matmul(
                            acc[:, ts(nh, 512)], lhsT, wv_s[:, ec, ts(nh, 512)],
                            start=(ec == 0), stop=(ec == EC - 1))
                stg = sb.tile([P, D], bf16, tag="vstg", bufs=4)
                nc.scalar.copy(stg, acc)
                nc.sync.dma_start(vin[ch][:, ot % 4, :], stg)
            nc.gpsimd.collective_compute(
                "AllGather", mybir.AluOpType.bypass,
                replica_groups=CC_GROUPS,
                ins=[vin[ch].opt()], outs=[vout[ch].opt()])
            for r in range(2):
                nc.sync.dma_start(V_s[:, 8 * r + 4 * ch: 8 * r + 4 * ch + 4, :],
                                  vout[ch][r])

        def q_proj_half(wq_s, nh):
            # Q^T for own query blocks 4nh..4nh+3
            for dc in range(EC):
                acch = ps.tile([P, 512], f32, tag="s")
                for ec in range(EC):
                    nc.tensor.matmul(
                        acch, wq_s[:, ec, ts(dc, P)], xoT_s[:, ec, ts(nh, 512)],
                        start=(ec == 0), stop=(ec == EC - 1))
                nc.scalar.copy(QT_s[:, dc, ts(nh, 512)], acch)

        k_proj_chunk(0)
        wv_s = load_w(wvT, "wv_s")
        v_proj_chunk(wv_s, 0)
        wq_s = load_w(wqT, "wq_s")
        q_proj_half(wq_s, 0)
        k_proj_chunk(1)
        v_proj_chunk(wv_s, 1)
        q_proj_half(wq_s, 1)

        # ---- attention, one 128-row query block at a time ----
        # Block j reads parity-p key tiles 0..j at KT_s cols [1024p, 1024p+L),
        # L=(j+1)*128; p_sb col-tile c<=j is parity-0 tile c (V slot c), else
        # parity-1 tile c-j-1 (V slot 8+c-j-1). Small blocks first so the
        # attention pipeline chases the arriving K/V AllGather chunks.
        inv_sqrt_d = 1.0 / float(np.sqrt(D))

        def s_phase(j):
            L = (j + 1) * P
            nch_r = (L + 511) // 512       # psum chunks per parity range
            p_sb = sb.tile([P, S], bf16, tag="p_sb", bufs=2)
            pT_sb = sb.tile([P, NKB, P], bf16, tag="pT_sb", bufs=2)
            dsl = sb.tile([P, 4], f32, tag="dsl", bufs=2)
            chg = 0
            for r in range(2):
                for ch in range(nch_r):
                    c0 = ch * 512
                    cw = min(512, L - c0)
                    sfull = ps.tile([P, 512], f32, tag="s")
                    sps = sfull[:, :cw]
                    for dc in range(EC):
                        nc.tensor.matmul(
                            sps, QT_s[:, dc, ts(j, P)],
                            KT_s[:, dc, r * 1024 + c0: r * 1024 + c0 + cw],
                            start=(dc == 0), stop=(dc == EC - 1))
                    if c0 + cw == L:  # last chunk of range r holds masked tile j
                        nc.vector.tensor_add(
                            sps[:, cw - P:cw], sps[:, cw - P:cw],
                            mask_s[:, j, ts(r, P)])
                    pcol = r * L + c0  # column inside p_sb
                    nc.scalar.activation(
                        p_sb[:, pcol:pcol + cw], sps,
                        mybir.ActivationFunctionType.Exp,
                        scale=inv_sqrt_d,
                        accum_out=dsl[:, chg:chg + 1])
                    # xbar-transpose the finished chunk off the hot engines:
                    # pT_sb[p, kt, q] = p_sb[q, 128*kt + p]
                    nc.sync.dma_start(pT_sb[:, pcol // P: (pcol + cw) // P, :],
                                      p_sb[:, pcol:pcol + cw], transpose=True)
                    chg += 1
            return p_sb, pT_sb, dsl, chg

        def av_phase(j, p_sb, pT_sb, dsl, nch):
            denom = sb.tile([P, 1], f32, tag="den", bufs=2)
            nc.vector.reduce_sum(denom, dsl[:, :nch], axis=mybir.AxisListType.X)
            recip = sb.tile([P, 1], f32, tag="rcp", bufs=2)
            nc.vector.reciprocal(recip, denom)

            nkt = 2 * (j + 1)
            acc = ps.tile([P, D], f32, tag="big")
            for c in range(nkt):
                slot = c if c <= j else 8 + (c - j - 1)
                for nh in range(2):
                    nc.tensor.matmul(
                        acc[:, ts(nh, 512)], pT_sb[:, c, :], V_s[:, slot, ts(nh, 512)],
                        start=(c == 0), stop=(c == nkt - 1))
            o_sb = sb.tile([P, D], f32, tag="o_sb", bufs=2)
            # normalize on DVE (idle now), halves so the out DMA overlaps
            nc.vector.tensor_scalar_mul(o_sb[:, :512], acc[:, :512], recip)
            nc.sync.dma_start(out[j, :, :512], o_sb[:, :512])
            nc.vector.tensor_scalar_mul(o_sb[:, 512:], acc[:, 512:], recip)
            nc.sync.dma_start(out[j, :, 512:], o_sb[:, 512:])

        pending = None
        for j in range(NQB):
            state = s_phase(j)
            if pending is not None:
                av_phase(*pending)
            pending = (j,) + state
        av_phase(*pending)


def build_nc(debug=False):
    nc = bacc.Bacc("TRN2", target_bir_lowering=False, debug=debug,
                   enable_asserts=False, num_devices=NCORES)
    bf16 = mybir.dt.bfloat16
    f32 = mybir.dt.float32
    xoT = nc.dram_tensor("xoT", (D, D), bf16, kind="ExternalInput").ap()
    wqT = nc.dram_tensor("wqT", (D, D), bf16, kind="ExternalInput").ap()
    wkT = nc.dram_tensor("wkT", (D, D), bf16, kind="ExternalInput").ap()
    wvT = nc.dram_tensor("wvT", (D, D), bf16, kind="ExternalInput").ap()
    maskadd = nc.dram_tensor("maskadd", (NQB, P, 2 * P), f32,
                             kind="ExternalInput").ap()
    out = nc.dram_tensor("out", (NQB, P, D), f32, kind="ExternalOutput").ap()
    with tile.TileContext(nc) as tc:
        _trace_kernel(tc, out, xoT, wqT, wkT, wvT, maskadd)
    nc.compile()
    return nc


def _get_compiled():
    global _compiled_nc
    if _compiled_nc is None:
        _compiled_nc = build_nc(debug=False)
    return _compiled_nc


def _get_runner():
    """Jit-once shard_map runner over the 8 NeuronCores.

    Mirrors bass2jax.run_bass_via_pjrt's multi-core branch, but caches the
    jitted executable so repeat kernel() calls skip retracing/recompiling.
    """
    global _runner
    if _runner is not None:
        return _runner
    import jax
    from jax.experimental.shard_map import shard_map
    from jax.sharding import Mesh, PartitionSpec
    from concourse import bass2jax

    nc = _get_compiled()
    bass2jax.install_neuronx_cc_hook()

    partition_name = (nc.partition_id_tensor.name
                      if nc.partition_id_tensor else None)
    in_names, out_names, out_avals, zero_outs = [], [], [], []
    for alloc in nc.m.functions[0].allocations:
        if not isinstance(alloc, mybir.MemoryLocationSet):
            continue
        name = alloc.memorylocations[0].name
        if alloc.kind == "ExternalInput":
            if name != partition_name:
                in_names.append(name)
        elif alloc.kind == "ExternalOutput":
            shape = tuple(alloc.tensor_shape)
            dtype = mybir.dt.np(alloc.dtype)
            out_names.append(name)
            out_avals.append(jax.core.ShapedArray(shape, dtype))
            zero_outs.append(np.zeros(shape, dtype))
    n_params = len(in_names)
    all_in_names = list(in_names) + list(out_names)
    if partition_name is not None:
        all_in_names.append(partition_name)
    donate = tuple(range(n_params, n_params + len(out_names)))

    def _body(*args):
        operands = list(args)
        if partition_name is not None:
            operands.append(bass2jax.partition_id_tensor())
        outs = bass2jax._bass_exec_p.bind(
            *operands,
            out_avals=tuple(out_avals),
            in_names=tuple(all_in_names),
            out_names=tuple(out_names),
            lowering_input_output_aliases=(),
            sim_require_finite=True,
            sim_require_nnan=True,
            nc=nc,
        )
        return tuple(outs)

    devices = jax.devices()[:NCORES]
    mesh = Mesh(np.asarray(devices), ("core",))
    nin = n_params + len(out_names)
    sharded = jax.jit(
        shard_map(_body, mesh=mesh,
                  in_specs=(PartitionSpec("core"),) * nin,
                  out_specs=(PartitionSpec("core"),) * len(out_names),
                  check_rep=False),
        donate_argnums=donate, keep_unused=True)
    _runner = (sharded, in_names, out_names, out_avals, n_params, zero_outs, mesh)
    return _runner


def run_device(in_maps):
    """Execute the compiled NEFF on all 8 cores; returns per-core output dicts."""
    sharded, in_names, out_names, out_avals, n_params, zero_outs, _ = _get_runner()
    concat_in = [
        np.concatenate([np.asarray(in_maps[c][nm]) for c in range(NCORES)], axis=0)
        for nm in in_names
    ]
    concat_zeros = [
        np.zeros((NCORES * z.shape[0], *z.shape[1:]), z.dtype) for z in zero_outs
    ]
    out_arrs = sharded(*concat_in, *concat_zeros)
    return [
        {nm: np.asarray(out_arrs[i]).reshape(NCORES, *out_avals[i].shape)[c]
         for i, nm in enumerate(out_names)}
        for c in range(NCORES)
    ]


def make_in_maps(x):
    """Per-core host-side slicing + layout prep (no matmul math here)."""
    x = np.asarray(x, dtype=np.float32)
    r = np.arange(P)
    tri_add = np.where(r[None, :] <= r[:, None], 0.0, MASKV).astype(np.float32)
    mask_h = []
    for h in range(2):
        if h == 0:
            blk = np.concatenate(
                [tri_add, np.full((P, P), MASKV, np.float32)], axis=1)
        else:
            blk = np.concatenate([np.zeros((P, P), np.float32), tri_add], axis=1)
        mask_h.append(np.ascontiguousarray(
            np.broadcast_to(blk, (NQB, P, 2 * P))).astype(np.float32))

    in_maps = []
    for c in range(NCORES):
        b, h = c // 2, c % 2
        blocks = [2 * j + h for j in range(NQB)]
        xo = np.concatenate([x[b][g * P:(g + 1) * P] for g in blocks], axis=0)
        xoT = np.ascontiguousarray(xo.T).astype(BF16)
        in_maps.append({
            "xoT": xoT,
            "maskadd": mask_h[h],
        })
    return in_maps


def make_weight_map(inputs):
    """Pre-transposed bf16 weights keyed by NEFF input name."""
    return {
        "wqT": np.ascontiguousarray(np.asarray(inputs["Wq"], np.float32).T).astype(BF16),
        "wkT": np.ascontiguousarray(np.asarray(inputs["Wk"], np.float32).T).astype(BF16),
        "wvT": np.ascontiguousarray(np.asarray(inputs["Wv"], np.float32).T).astype(BF16),
    }


def kernel(x, Wq, bq, Wk, bk, Wv, bv, mask):
    global last_result
    x = np.asarray(x, np.float32)
    Wq = np.asarray(Wq, np.float32)
    Wk = np.asarray(Wk, np.float32)
    Wv = np.asarray(Wv, np.float32)
    bq = np.asarray(bq, np.float32)
    bk = np.asarray(bk, np.float32)
    bv = np.asarray(bv, np.float32)
    mask = np.asarray(mask)

    causal = bool(np.array_equal(mask != 0, np.tril(np.ones(mask.shape, bool))))
    if np.any(bq) or np.any(bk) or not causal:
        return _np_reference(x, Wq, bq, Wk, bk, Wv, bv, mask)

    in_maps = make_in_maps(x)
    wT = make_weight_map({"Wq": Wq, "Wk": Wk, "Wv": Wv})
    for m in in_maps:
        m.update(wT)

    results = None
    for attempt in range(3):  # remote NeuronCores occasionally wedge transiently
        try:
            results = run_device(in_maps)
            break
        except Exception:
            if attempt == 2:
                raise
            time.sleep(30)

    out = np.empty((B * S, D), np.float32)
    for c in range(NCORES):
        b, h = c // 2, c % 2
        o = np.asarray(results[c]["out"], np.float32)
        for j in range(NQB):
            g = 2 * j + h
            out[b * S + g * P: b * S + (g + 1) * P] = o[j]
    if np.any(bv):
        out = out + bv[None, :]  # attn rows sum to 1, so bv adds exactly
    return out


def _np_reference(x, Wq, bq, Wk, bk, Wv, bv, mask):
    outs = []
    for b in range(x.shape[0]):
        xb = x[b]
        Q = xb @ Wq.T + bq
        K = xb @ Wk.T + bk
        V = xb @ Wv.T + bv
        Sc = (Q @ K.T) / np.float32(np.sqrt(x.shape[2]))
        Sc = np.where(mask == 0, np.float32(-1e9), Sc)
        Sc = Sc - Sc.max(axis=1, keepdims=True)
        E = np.exp(Sc)
        A = E / E.sum(axis=1, keepdims=True)
        outs.append(A @ V)
    return np.concatenate(outs, axis=0).astype(np.float32)
